# revision 2
# baseline (speedup 1.0000x reference)
"""Multi-head attention (B=4, T=2048, D=1024, H=16) on 8 Trainium2 cores.

Sharding: core c handles (batch b = c//2, head-group g = c%2) — 8 heads,
512 output features. No inter-core communication.

Host-side: rows of K/V masked out by mask_attn and rows of Q masked out by
mask_out are compacted away (their probabilities / outputs are exactly zero
in the reference), then padded to a multiple of 128. Activations and
weight slices are pre-transposed so every device matmul contracts over the
partition dim. Device returns per-head unnormalized PV accumulations plus
the softmax denominator (ones-column trick); host divides and scatters.
"""

import os
import sys

sys.path.insert(0, "/opt/trn_rl_repo")

import numpy as np
from contextlib import ExitStack

import concourse.bacc as bacc
import concourse.tile as tile
from concourse import mybir
from concourse.bass_utils import run_bass_kernel_spmd

F32 = mybir.dt.float32
F32R = mybir.dt.float32r

HID = 1024
FO = 512          # projection features per core = 8 heads * 64
HPC = 8           # heads per core
DH = 64
NFI = HID // 128  # contraction chunks
N_CORES = 8


def _tiles(total, w):
    out = []
    o = 0
    while o < total:
        tw = min(w, total - o)
        out.append((o, tw))
        o += tw
    return out


def _build(TQ, TK):
    NTK = TK // 128
    TQT = _tiles(TQ, 512)

    nc = bacc.Bacc("TRN2", target_bir_lowering=False, debug=False)

    qT_d = nc.declare_dram_parameter("qT", [HID, TQ], F32R, isOutput=False)
    kT_d = nc.declare_dram_parameter("kT", [HID, TK], F32R, isOutput=False)
    vT_d = nc.declare_dram_parameter("vT", [HID, TK], F32R, isOutput=False)
    wqT_d = nc.declare_dram_parameter("wqT", [HID, FO], F32R, isOutput=False)
    wkT_d = nc.declare_dram_parameter("wkT", [HID, FO], F32R, isOutput=False)
    wvT_d = nc.declare_dram_parameter("wvT", [HID, FO], F32R, isOutput=False)
    bq_d = nc.declare_dram_parameter("bq", [1, FO], F32R, isOutput=False)
    bk_d = nc.declare_dram_parameter("bk", [1, FO], F32R, isOutput=False)
    bv_d = nc.declare_dram_parameter("bv", [1, FO], F32R, isOutput=False)
    biask_d = nc.declare_dram_parameter("biask", [128, NTK], F32, isOutput=False)
    ones1_d = nc.declare_dram_parameter("ones1", [1, 512], F32R, isOutput=False)
    onesv_d = nc.declare_dram_parameter("onesv", [128, NTK * HPC], F32R, isOutput=False)
    out_d = nc.declare_dram_parameter("out", [65, HPC, TQ], F32, isOutput=True)

    Exp = mybir.ActivationFunctionType.Exp

    with tile.TileContext(nc) as tc, ExitStack() as ctx:
        res = ctx.enter_context(tc.tile_pool(name="res", bufs=1))
        qhT = res.tile([128, 4, TQ], F32R)      # [fo%128, fo//128, t]
        khT = res.tile([128, 4, TK], F32R)
        vh = res.tile([128, NTK, HPC, 65], F32R)  # [t%128, t//128, head, dh+1]
        ones = res.tile([1, 512], F32R)
        biask_sb = res.tile([128, NTK], F32)
        bq_sb = res.tile([1, FO], F32R)
        bk_sb = res.tile([1, FO], F32R)
        bv_sb = res.tile([1, FO], F32R)

        nc.sync.dma_start(ones[:], ones1_d[:])
        nc.sync.dma_start(vh[:, :, :, 64:65], onesv_d[:])
        nc.sync.dma_start(biask_sb[:], biask_d[:])
        nc.sync.dma_start(bq_sb[:], bq_d[:])
        nc.sync.dma_start(bk_sb[:], bk_d[:])
        nc.sync.dma_start(bv_sb[:], bv_d[:])

        # ---------------- projections ----------------
        with (
            tc.tile_pool(name="wpool", bufs=1) as wpool,
            tc.tile_pool(name="vres", bufs=1) as vres,
            tc.tile_pool(name="astream", bufs=4) as astream,
        ):
            wq_sb = wpool.tile([128, NFI, FO], F32R)
            wk_sb = wpool.tile([128, NFI, FO], F32R)
            wv_sb = wpool.tile([128, NFI, FO], F32R)
            nc.sync.dma_start(wq_sb[:], wqT_d.rearrange("(c p) n -> p c n", p=128))
            nc.sync.dma_start(wk_sb[:], wkT_d.rearrange("(c p) n -> p c n", p=128))
            nc.sync.dma_start(wv_sb[:], wvT_d.rearrange("(c p) n -> p c n", p=128))
            vT_sb = vres.tile([128, NFI, TK], F32R)
            nc.sync.dma_start(vT_sb[:], vT_d.rearrange("(c p) t -> p c t", p=128))

            # K then Q projections, transposed layout [fo, t]
            with tc.tile_pool(name="ppqk", bufs=2, space="PSUM") as ppqk:
                for src_d, w_sb, b_sb, dst, nT in (
                    (kT_d, wk_sb, bk_sb, khT, TK),
                    (qT_d, wq_sb, bq_sb, qhT, TQ),
                ):
                    for (t0, tw) in _tiles(nT, 512):
                        ps = ppqk.tile([128, 4, 512], F32, name="qkps")
                        for c in range(NFI):
                            xs = astream.tile([128, 512], F32R, name="xs")
                            nc.sync.dma_start(
                                xs[:, :tw], src_d[c * 128:(c + 1) * 128, t0:t0 + tw]
                            )
                            for jf in range(4):
                                nc.tensor.matmul(
                                    ps[:, jf, :tw],
                                    w_sb[:, c, jf * 128:(jf + 1) * 128],
                                    xs[:, :tw],
                                    start=(c == 0),
                                    stop=False,
                                )
                        for jf in range(4):
                            nc.tensor.matmul(
                                ps[:, jf, :tw],
                                b_sb[0:1, jf * 128:(jf + 1) * 128],
                                ones[0:1, :tw],
                                start=False,
                                stop=True,
                            )
                        nc.vector.tensor_copy(dst[:, :, t0:t0 + tw], ps[:, :, :tw])

            # V projection, natural layout [t, fo]
            with tc.tile_pool(name="ppv", bufs=4, space="PSUM") as ppv:
                for it in range(NTK):
                    psv = ppv.tile([128, FO], F32, name="vps")
                    for c in range(NFI):
                        nc.tensor.matmul(
                            psv[:],
                            vT_sb[:, c, it * 128:(it + 1) * 128],
                            wv_sb[:, c, :],
                            start=(c == 0),
                            stop=False,
                        )
                    nc.tensor.matmul(
                        psv[:], ones[0:1, 0:128], bv_sb[:], start=False, stop=True
                    )
                    nc.vector.tensor_copy(
                        vh[:, it, :, 0:64],
                        psv[:].rearrange("p (h d) -> p h d", h=HPC),
                    )

        # ---------------- attention ----------------
        with (
            tc.tile_pool(name="scps", bufs=3, space="PSUM") as scps,
            tc.tile_pool(name="otps", bufs=1, space="PSUM") as otps,
            tc.tile_pool(name="probs", bufs=3) as probs_pool,
            tc.tile_pool(name="park", bufs=4) as park_pool,
        ):
            for j in range(4):  # head pair: local heads 2j, 2j+1
                for (t0, tw) in TQT:
                    o0 = otps.tile([65, 512], F32, name="ot0")
                    o1 = otps.tile([65, 512], F32, name="ot1")
                    for it in range(NTK):
                        sp = scps.tile([128, 2, 512], F32, name="sc")
                        nc.tensor.matmul(
                            sp[:, 0, :tw],
                            khT[0:64, j, it * 128:(it + 1) * 128],
                            qhT[0:64, j, t0:t0 + tw],
                            start=True, stop=True,
                        )
                        nc.tensor.matmul(
                            sp[:, 1, :tw],
                            khT[64:128, j, it * 128:(it + 1) * 128],
                            qhT[64:128, j, t0:t0 + tw],
                            start=True, stop=True,
                        )
                        pr = probs_pool.tile([128, 2, 512], F32R, name="pr")
                        nc.scalar.activation(
                            pr[:, :, :tw], sp[:, :, :tw], Exp,
                            bias=biask_sb[:, it:it + 1], scale=0.125,
                        )
                        nc.tensor.matmul(
                            o0[:, :tw], vh[:, it, 2 * j, :], pr[:, 0, :tw],
                            start=(it == 0), stop=(it == NTK - 1),
                        )
                        nc.tensor.matmul(
                            o1[:, :tw], vh[:, it, 2 * j + 1, :], pr[:, 1, :tw],
                            start=(it == 0), stop=(it == NTK - 1),
                        )
                    pk0 = park_pool.tile([65, 512], F32, name="pk")
                    nc.vector.tensor_copy(pk0[:, :tw], o0[:, :tw])
                    nc.sync.dma_start(out_d[:, 2 * j, t0:t0 + tw], pk0[:, :tw])
                    pk1 = park_pool.tile([65, 512], F32, name="pk")
                    nc.vector.tensor_copy(pk1[:, :tw], o1[:, :tw])
                    nc.sync.dma_start(out_d[:, 2 * j + 1, t0:t0 + tw], pk1[:, :tw])

    nc.finalize()
    return nc


def kernel(q, k, v, Wq, bq, Wk, bk, Wv, bv, mask_attn, mask_out):
    q = np.asarray(q, np.float32)
    k = np.asarray(k, np.float32)
    v = np.asarray(v, np.float32)
    Wq = np.asarray(Wq, np.float32)
    Wk = np.asarray(Wk, np.float32)
    Wv = np.asarray(Wv, np.float32)
    bq = np.asarray(bq, np.float32)
    bk = np.asarray(bk, np.float32)
    bv = np.asarray(bv, np.float32)
    mask_attn = np.asarray(mask_attn)
    mask_out = np.asarray(mask_out)

    B, T, _ = q.shape
    idxk = [np.flatnonzero(mask_attn[b]) for b in range(B)]
    idxq = [np.flatnonzero(mask_out[b]) for b in range(B)]
    TK = max(128, -(-max(len(i) for i in idxk) // 128) * 128)
    TQ = max(128, -(-max(len(i) for i in idxq) // 128) * 128)
    NTK = TK // 128

    nc = _build(TQ, TK)

    in_maps = []
    for c in range(N_CORES):
        b, g = c // 2, c % 2
        sl = slice(g * FO, (g + 1) * FO)
        nk, nq = len(idxk[b]), len(idxq[b])
        qc = np.zeros((TQ, HID), np.float32)
        qc[:nq] = q[b][idxq[b]]
        kc = np.zeros((TK, HID), np.float32)
        kc[:nk] = k[b][idxk[b]]
        vc = np.zeros((TK, HID), np.float32)
        vc[:nk] = v[b][idxk[b]]
        biask = np.full(TK, -30000.0, np.float32)
        biask[:nk] = 0.0
        in_maps.append({
            "qT": np.ascontiguousarray(qc.T),
            "kT": np.ascontiguousarray(kc.T),
            "vT": np.ascontiguousarray(vc.T),
            "wqT": np.ascontiguousarray(Wq[sl].T),
            "wkT": np.ascontiguousarray(Wk[sl].T),
            "wvT": np.ascontiguousarray(Wv[sl].T),
            "bq": np.ascontiguousarray(bq[sl].reshape(1, FO)),
            "bk": np.ascontiguousarray(bk[sl].reshape(1, FO)),
            "bv": np.ascontiguousarray(bv[sl].reshape(1, FO)),
            "biask": np.ascontiguousarray(biask.reshape(NTK, 128).T),
            "ones1": np.ones((1, 512), np.float32),
            "onesv": np.ones((128, NTK * HPC), np.float32),
        })

    trace_dir = os.environ.get("KERNEL_TRACE_DIR")
    if trace_dir:
        res = run_bass_kernel_spmd(
            nc, in_maps, list(range(N_CORES)), trace=True, tmpdir=trace_dir
        )
        print(f"HW exec time: {res.exec_time_ns} ns")
    else:
        res = run_bass_kernel_spmd(nc, in_maps, list(range(N_CORES)))

    out_full = np.zeros((B, T, HID), np.float32)
    for c in range(N_CORES):
        b, g = c // 2, c % 2
        nq = len(idxq[b])
        u = res.results[c]["out"]  # [65, HPC, TQ]
        o = u[:64, :, :nq] / u[64:65, :, :nq]
        o = o.transpose(2, 1, 0).reshape(nq, FO)
        out_full[b, idxq[b], g * FO:(g + 1) * FO] = o
    return out_full


# revision 3
# speedup vs baseline: 1.3120x; 1.3120x over previous
"""Multi-head attention (B=4, T=2048, D=1024, H=16) on 8 Trainium2 cores.

Sharding: core c handles (batch b = c//2, head-group g = c%2) — 8 heads,
512 output features. No inter-core communication.

Host-side: rows of K/V masked out by mask_attn and rows of Q masked out by
mask_out are compacted away (their probabilities / outputs are exactly zero
in the reference), then padded to a multiple of 128. Activations and
weight slices are pre-transposed so every device matmul contracts over the
partition dim. Device returns per-head unnormalized PV accumulations plus
the softmax denominator (ones-column trick); host divides and scatters.
"""

import os
import sys

sys.path.insert(0, "/opt/trn_rl_repo")

import numpy as np
import ml_dtypes
from contextlib import ExitStack

import concourse.bacc as bacc
import concourse.tile as tile
from concourse import mybir
from concourse.bass_utils import run_bass_kernel_spmd

F32 = mybir.dt.float32
F32R = mybir.dt.float32r
BF16 = mybir.dt.bfloat16

HID = 1024
FO = 512          # projection features per core = 8 heads * 64
HPC = 8           # heads per core
DH = 64
NFI = HID // 128  # contraction chunks
N_CORES = 8


def _tiles(total, w):
    out = []
    o = 0
    while o < total:
        tw = min(w, total - o)
        out.append((o, tw))
        o += tw
    return out


def _build(TQ, TK):
    NTK = TK // 128
    TQT = _tiles(TQ, 512)

    nc = bacc.Bacc("TRN2", target_bir_lowering=False, debug=False)

    qT_d = nc.declare_dram_parameter("qT", [HID, TQ], BF16, isOutput=False)
    kT_d = nc.declare_dram_parameter("kT", [HID, TK], BF16, isOutput=False)
    vT_d = nc.declare_dram_parameter("vT", [HID, TK], BF16, isOutput=False)
    wqT_d = nc.declare_dram_parameter("wqT", [HID, FO], BF16, isOutput=False)
    wkT_d = nc.declare_dram_parameter("wkT", [HID, FO], BF16, isOutput=False)
    wvT_d = nc.declare_dram_parameter("wvT", [HID, FO], BF16, isOutput=False)
    bq_d = nc.declare_dram_parameter("bq", [1, FO], BF16, isOutput=False)
    bk_d = nc.declare_dram_parameter("bk", [1, FO], BF16, isOutput=False)
    bv_d = nc.declare_dram_parameter("bv", [1, FO], BF16, isOutput=False)
    biask_d = nc.declare_dram_parameter("biask", [128, NTK], F32, isOutput=False)
    ones1_d = nc.declare_dram_parameter("ones1", [1, 512], BF16, isOutput=False)
    onesv_d = nc.declare_dram_parameter("onesv", [128, NTK * HPC], BF16, isOutput=False)
    out_d = nc.declare_dram_parameter("out", [65, HPC, TQ], F32, isOutput=True)

    Exp = mybir.ActivationFunctionType.Exp

    with tile.TileContext(nc) as tc, ExitStack() as ctx:
        res = ctx.enter_context(tc.tile_pool(name="res", bufs=1))
        qhT = res.tile([128, 4, TQ], BF16)      # [fo%128, fo//128, t]
        khT = res.tile([128, 4, TK], BF16)
        vh = res.tile([128, NTK, HPC, 65], BF16)  # [t%128, t//128, head, dh+1]
        ones = res.tile([1, 512], BF16)
        biask_sb = res.tile([128, NTK], F32)
        bq_sb = res.tile([1, FO], BF16)
        bk_sb = res.tile([1, FO], BF16)
        bv_sb = res.tile([1, FO], BF16)

        nc.gpsimd.dma_start(ones[:], ones1_d[:])
        nc.gpsimd.dma_start(vh[:, :, :, 64:65], onesv_d[:])
        nc.gpsimd.dma_start(biask_sb[:], biask_d[:])
        nc.gpsimd.dma_start(bq_sb[:], bq_d[:])
        nc.gpsimd.dma_start(bk_sb[:], bk_d[:])
        nc.gpsimd.dma_start(bv_sb[:], bv_d[:])

        # ---------------- projections ----------------
        with (
            tc.tile_pool(name="wpool", bufs=1) as wpool,
            tc.tile_pool(name="vres", bufs=1) as vres,
            tc.tile_pool(name="astream", bufs=4) as astream,
        ):
            wq_sb = wpool.tile([128, NFI, FO], BF16)
            wk_sb = wpool.tile([128, NFI, FO], BF16)
            wv_sb = wpool.tile([128, NFI, FO], BF16)
            nc.sync.dma_start(wq_sb[:], wqT_d.rearrange("(c p) n -> p c n", p=128))
            nc.sync.dma_start(wk_sb[:], wkT_d.rearrange("(c p) n -> p c n", p=128))
            nc.gpsimd.dma_start(wv_sb[:], wvT_d.rearrange("(c p) n -> p c n", p=128))
            vT_sb = vres.tile([128, NFI, TK], BF16)
            nc.gpsimd.dma_start(vT_sb[:], vT_d.rearrange("(c p) t -> p c t", p=128))

            # K then Q projections, transposed layout [fo, t]
            with tc.tile_pool(name="ppqk", bufs=2, space="PSUM") as ppqk:
                for src_d, w_sb, b_sb, dst, nT in (
                    (kT_d, wk_sb, bk_sb, khT, TK),
                    (qT_d, wq_sb, bq_sb, qhT, TQ),
                ):
                    for (t0, tw) in _tiles(nT, 512):
                        ps = ppqk.tile([128, 4, 512], F32, name="qkps")
                        for c in range(NFI):
                            xs = astream.tile([128, 512], BF16, name="xs")
                            nc.sync.dma_start(
                                xs[:, :tw], src_d[c * 128:(c + 1) * 128, t0:t0 + tw]
                            )
                            for jf in range(4):
                                nc.tensor.matmul(
                                    ps[:, jf, :tw],
                                    w_sb[:, c, jf * 128:(jf + 1) * 128],
                                    xs[:, :tw],
                                    start=(c == 0),
                                    stop=False,
                                )
                        for jf in range(4):
                            nc.tensor.matmul(
                                ps[:, jf, :tw],
                                b_sb[0:1, jf * 128:(jf + 1) * 128],
                                ones[0:1, :tw],
                                start=False,
                                stop=True,
                            )
                        nc.vector.tensor_copy(dst[:, :, t0:t0 + tw], ps[:, :, :tw])

            # V projection, natural layout [t, fo]
            with tc.tile_pool(name="ppv", bufs=4, space="PSUM") as ppv:
                for it in range(NTK):
                    psv = ppv.tile([128, FO], F32, name="vps")
                    for c in range(NFI):
                        nc.tensor.matmul(
                            psv[:],
                            vT_sb[:, c, it * 128:(it + 1) * 128],
                            wv_sb[:, c, :],
                            start=(c == 0),
                            stop=False,
                        )
                    nc.tensor.matmul(
                        psv[:], ones[0:1, 0:128], bv_sb[:], start=False, stop=True
                    )
                    nc.vector.tensor_copy(
                        vh[:, it, :, 0:64],
                        psv[:].rearrange("p (h d) -> p h d", h=HPC),
                    )

        # ---------------- attention ----------------
        with (
            tc.tile_pool(name="scps", bufs=3, space="PSUM") as scps,
            tc.tile_pool(name="otps", bufs=1, space="PSUM") as otps,
            tc.tile_pool(name="probs", bufs=3) as probs_pool,
            tc.tile_pool(name="park", bufs=4) as park_pool,
        ):
            for j in range(4):  # head pair: local heads 2j, 2j+1
                for (t0, tw) in TQT:
                    o0 = otps.tile([65, 512], F32, name="ot0")
                    o1 = otps.tile([65, 512], F32, name="ot1")
                    for it in range(NTK):
                        sp = scps.tile([128, 2, 512], F32, name="sc")
                        nc.tensor.matmul(
                            sp[:, 0, :tw],
                            khT[0:64, j, it * 128:(it + 1) * 128],
                            qhT[0:64, j, t0:t0 + tw],
                            start=True, stop=True,
                        )
                        nc.tensor.matmul(
                            sp[:, 1, :tw],
                            khT[64:128, j, it * 128:(it + 1) * 128],
                            qhT[64:128, j, t0:t0 + tw],
                            start=True, stop=True,
                        )
                        pr = probs_pool.tile([128, 2, 512], BF16, name="pr")
                        nc.scalar.activation(
                            pr[:, :, :tw], sp[:, :, :tw], Exp,
                            bias=biask_sb[:, it:it + 1], scale=0.125,
                        )
                        nc.tensor.matmul(
                            o0[:, :tw], vh[:, it, 2 * j, :], pr[:, 0, :tw],
                            start=(it == 0), stop=(it == NTK - 1),
                        )
                        nc.tensor.matmul(
                            o1[:, :tw], vh[:, it, 2 * j + 1, :], pr[:, 1, :tw],
                            start=(it == 0), stop=(it == NTK - 1),
                        )
                    pk0 = park_pool.tile([65, 512], F32, name="pk")
                    nc.vector.tensor_copy(pk0[:, :tw], o0[:, :tw])
                    nc.sync.dma_start(out_d[:, 2 * j, t0:t0 + tw], pk0[:, :tw])
                    pk1 = park_pool.tile([65, 512], F32, name="pk")
                    nc.vector.tensor_copy(pk1[:, :tw], o1[:, :tw])
                    nc.sync.dma_start(out_d[:, 2 * j + 1, t0:t0 + tw], pk1[:, :tw])

    nc.finalize()
    return nc


def kernel(q, k, v, Wq, bq, Wk, bk, Wv, bv, mask_attn, mask_out):
    q = np.asarray(q, np.float32)
    k = np.asarray(k, np.float32)
    v = np.asarray(v, np.float32)
    Wq = np.asarray(Wq, np.float32)
    Wk = np.asarray(Wk, np.float32)
    Wv = np.asarray(Wv, np.float32)
    bq = np.asarray(bq, np.float32)
    bk = np.asarray(bk, np.float32)
    bv = np.asarray(bv, np.float32)
    mask_attn = np.asarray(mask_attn)
    mask_out = np.asarray(mask_out)

    B, T, _ = q.shape
    idxk = [np.flatnonzero(mask_attn[b]) for b in range(B)]
    idxq = [np.flatnonzero(mask_out[b]) for b in range(B)]
    TK = max(128, -(-max(len(i) for i in idxk) // 128) * 128)
    TQ = max(128, -(-max(len(i) for i in idxq) // 128) * 128)
    NTK = TK // 128

    nc = _build(TQ, TK)

    in_maps = []
    for c in range(N_CORES):
        b, g = c // 2, c % 2
        sl = slice(g * FO, (g + 1) * FO)
        nk, nq = len(idxk[b]), len(idxq[b])
        qc = np.zeros((TQ, HID), np.float32)
        qc[:nq] = q[b][idxq[b]]
        kc = np.zeros((TK, HID), np.float32)
        kc[:nk] = k[b][idxk[b]]
        vc = np.zeros((TK, HID), np.float32)
        vc[:nk] = v[b][idxk[b]]
        biask = np.full(TK, -30000.0, np.float32)
        biask[:nk] = 0.0
        in_maps.append({
            "qT": np.ascontiguousarray(qc.T).astype(ml_dtypes.bfloat16),
            "kT": np.ascontiguousarray(kc.T).astype(ml_dtypes.bfloat16),
            "vT": np.ascontiguousarray(vc.T).astype(ml_dtypes.bfloat16),
            "wqT": np.ascontiguousarray(Wq[sl].T).astype(ml_dtypes.bfloat16),
            "wkT": np.ascontiguousarray(Wk[sl].T).astype(ml_dtypes.bfloat16),
            "wvT": np.ascontiguousarray(Wv[sl].T).astype(ml_dtypes.bfloat16),
            "bq": bq[sl].reshape(1, FO).astype(ml_dtypes.bfloat16),
            "bk": bk[sl].reshape(1, FO).astype(ml_dtypes.bfloat16),
            "bv": bv[sl].reshape(1, FO).astype(ml_dtypes.bfloat16),
            "biask": np.ascontiguousarray(biask.reshape(NTK, 128).T),
            "ones1": np.ones((1, 512), ml_dtypes.bfloat16),
            "onesv": np.ones((128, NTK * HPC), ml_dtypes.bfloat16),
        })

    trace_dir = os.environ.get("KERNEL_TRACE_DIR")
    if trace_dir:
        res = run_bass_kernel_spmd(
            nc, in_maps, list(range(N_CORES)), trace=True, tmpdir=trace_dir
        )
        print(f"HW exec time: {res.exec_time_ns} ns")
    else:
        res = run_bass_kernel_spmd(nc, in_maps, list(range(N_CORES)))

    out_full = np.zeros((B, T, HID), np.float32)
    for c in range(N_CORES):
        b, g = c // 2, c % 2
        nq = len(idxq[b])
        u = res.results[c]["out"]  # [65, HPC, TQ]
        o = u[:64, :, :nq] / u[64:65, :, :nq]
        o = o.transpose(2, 1, 0).reshape(nq, FO)
        out_full[b, idxq[b], g * FO:(g + 1) * FO] = o
    return out_full


# revision 4
# speedup vs baseline: 1.3551x; 1.0328x over previous
"""Multi-head attention (B=4, T=2048, D=1024, H=16) on 8 Trainium2 cores.

Sharding: core c handles (batch b = c//2, head-group g = c%2) — 8 heads,
512 output features. No inter-core communication.

Host-side: rows of K/V masked out by mask_attn and rows of Q masked out by
mask_out are compacted away (their probabilities / outputs are exactly zero
in the reference), then padded to a multiple of 128. Activations and
weight slices are pre-transposed so every device matmul contracts over the
partition dim, and converted to bf16 (PSUM accumulation is fp32).

Device per core: project K/Q into transposed [feature, token] layout and V
into natural [token, feature] layout (biases added via K=1 ones-matmuls);
scores^T = K_h @ Q_h^T per head pair, packed into disjoint PE row groups;
one ScalarE instruction applies scale + key-padding bias + exp per 2-head
PSUM tile; PV accumulates [V_h | 1]^T @ probs^T giving the output and the
softmax denominator (ones column). Projections of head-pair j+1 are
emission-interleaved into pair j's ACT-bound attention loop to fill PE
idle slots. Host divides by the denominator and scatters rows.
"""

import itertools
import os
import sys

sys.path.insert(0, "/opt/trn_rl_repo")

import numpy as np
import ml_dtypes
from contextlib import ExitStack

import concourse.bacc as bacc
import concourse.tile as tile
from concourse import mybir
from concourse.bass_utils import run_bass_kernel_spmd

F32 = mybir.dt.float32
BF16 = mybir.dt.bfloat16

HID = 1024
FO = 512          # projection features per core = 8 heads * 64
HPC = 8           # heads per core
NFI = HID // 128  # contraction chunks
N_CORES = 8


def _tiles(total, w):
    out = []
    o = 0
    while o < total:
        tw = min(w, total - o)
        out.append((o, tw))
        o += tw
    return out


def _build(TQ, TK):
    NTK = TK // 128
    TQT = _tiles(TQ, 512)

    nc = bacc.Bacc("TRN2", target_bir_lowering=False, debug=False)

    qT_d = nc.declare_dram_parameter("qT", [HID, TQ], BF16, isOutput=False)
    kT_d = nc.declare_dram_parameter("kT", [HID, TK], BF16, isOutput=False)
    vT_d = nc.declare_dram_parameter("vT", [HID, TK], BF16, isOutput=False)
    wqT_d = nc.declare_dram_parameter("wqT", [HID, FO], BF16, isOutput=False)
    wkT_d = nc.declare_dram_parameter("wkT", [HID, FO], BF16, isOutput=False)
    wvT_d = nc.declare_dram_parameter("wvT", [HID, FO], BF16, isOutput=False)
    bq_d = nc.declare_dram_parameter("bq", [1, FO], BF16, isOutput=False)
    bk_d = nc.declare_dram_parameter("bk", [1, FO], BF16, isOutput=False)
    bv_d = nc.declare_dram_parameter("bv", [1, FO], BF16, isOutput=False)
    biask_d = nc.declare_dram_parameter("biask", [128, NTK], F32, isOutput=False)
    ones1_d = nc.declare_dram_parameter("ones1", [1, 512], BF16, isOutput=False)
    onesv_d = nc.declare_dram_parameter("onesv", [128, NTK * HPC], BF16, isOutput=False)
    out_d = nc.declare_dram_parameter("out", [65, HPC, TQ], F32, isOutput=True)

    Exp = mybir.ActivationFunctionType.Exp

    with tile.TileContext(nc) as tc, ExitStack() as ctx:
        res = ctx.enter_context(tc.tile_pool(name="res", bufs=1))
        qhT = res.tile([128, 4, TQ], BF16)        # [fo%128, pair, t]
        khT = res.tile([128, 4, TK], BF16)
        vh = res.tile([128, NTK, HPC, 65], BF16)  # [t%128, t//128, head, dh+1]
        ones = res.tile([1, 512], BF16)
        biask_sb = res.tile([128, NTK], F32)
        bq_sb = res.tile([1, FO], BF16)
        bk_sb = res.tile([1, FO], BF16)
        bv_sb = res.tile([1, FO], BF16)
        kT_sb = res.tile([128, NFI, TK], BF16)
        qT_sb = res.tile([128, NFI, TQ], BF16)
        vT_sb = res.tile([128, NFI, TK], BF16)
        wq_sb = res.tile([128, NFI, FO], BF16)
        wk_sb = res.tile([128, NFI, FO], BF16)
        wv_sb = res.tile([128, NFI, FO], BF16)

        # Constants + V-path via gpsimd SWDGE; K-path on the sync ring;
        # vT/qT on the scalar HWDGE ring (idle until attention starts).
        nc.gpsimd.dma_start(biask_sb[:], biask_d[:])
        nc.gpsimd.dma_start(bk_sb[:], bk_d[:])
        nc.gpsimd.dma_start(bq_sb[:], bq_d[:])
        nc.gpsimd.dma_start(bv_sb[:], bv_d[:])
        nc.gpsimd.dma_start(ones[:], ones1_d[:])
        nc.gpsimd.dma_start(vh[:, :, :, 64:65], onesv_d[:])
        nc.gpsimd.dma_start(wv_sb[:], wvT_d.rearrange("(c p) n -> p c n", p=128))
        nc.sync.dma_start(wk_sb[:], wkT_d.rearrange("(c p) n -> p c n", p=128))
        nc.sync.dma_start(kT_sb[:], kT_d.rearrange("(c p) t -> p c t", p=128))
        nc.sync.dma_start(wq_sb[:], wqT_d.rearrange("(c p) n -> p c n", p=128))
        nc.scalar.dma_start(vT_sb[:], vT_d.rearrange("(c p) t -> p c t", p=128))
        nc.scalar.dma_start(qT_sb[:], qT_d.rearrange("(c p) t -> p c t", p=128))

        ppj = ctx.enter_context(tc.tile_pool(name="ppj", bufs=2, space="PSUM"))
        scps = ctx.enter_context(tc.tile_pool(name="scps", bufs=2, space="PSUM"))
        otps = ctx.enter_context(tc.tile_pool(name="otps", bufs=1, space="PSUM"))
        probs_pool = ctx.enter_context(tc.tile_pool(name="probs", bufs=3))
        park_pool = ctx.enter_context(tc.tile_pool(name="park", bufs=4))

        def gen_kq_proj(jf, src_sb, w_sb, b_sb, dst, nT):
            """Projection of feature tile jf (one head pair), [fo, t] layout."""
            for (t0, tw) in _tiles(nT, 512):
                ps = ppj.tile([128, 512], F32, name="pjps")
                for c in range(NFI):
                    nc.tensor.matmul(
                        ps[:, :tw],
                        w_sb[:, c, jf * 128:(jf + 1) * 128],
                        src_sb[:, c, t0:t0 + tw],
                        start=(c == 0), stop=False,
                    )
                    if c % 3 == 2:
                        yield
                nc.tensor.matmul(
                    ps[:, :tw],
                    b_sb[0:1, jf * 128:(jf + 1) * 128],
                    ones[0:1, :tw],
                    start=False, stop=True,
                )
                nc.vector.tensor_copy(dst[:, jf, t0:t0 + tw], ps[:, :tw])
                yield

        def gen_v_proj(half):
            """V projection for heads 4*half .. 4*half+3, natural layout."""
            f0 = half * 256
            for it in range(NTK):
                ps = ppj.tile([128, 512], F32, name="pjps")
                for c in range(NFI):
                    nc.tensor.matmul(
                        ps[:, :256],
                        vT_sb[:, c, it * 128:(it + 1) * 128],
                        wv_sb[:, c, f0:f0 + 256],
                        start=(c == 0), stop=False,
                    )
                    if c % 4 == 3:
                        yield
                nc.tensor.matmul(
                    ps[:, :256], ones[0:1, 0:128], bv_sb[0:1, f0:f0 + 256],
                    start=False, stop=True,
                )
                nc.vector.tensor_copy(
                    vh[:, it, 4 * half:4 * half + 4, 0:64],
                    ps[:, :256].rearrange("p (h d) -> p h d", h=4),
                )
                yield

        def drain(g):
            for _ in g:
                pass

        # Upfront: everything head pairs 0 (K, Q) and 0-1 (V) need.
        drain(gen_kq_proj(0, kT_sb, wk_sb, bk_sb, khT, TK))
        drain(gen_v_proj(0))
        drain(gen_kq_proj(0, qT_sb, wq_sb, bq_sb, qhT, TQ))

        for j in range(4):  # head pair: local heads 2j, 2j+1
            if j < 3:
                gens = [
                    gen_kq_proj(j + 1, kT_sb, wk_sb, bk_sb, khT, TK),
                    gen_kq_proj(j + 1, qT_sb, wq_sb, bq_sb, qhT, TQ),
                ]
                if j == 1:
                    gens.insert(0, gen_v_proj(1))
                nxt = itertools.chain(*gens)
            else:
                nxt = iter(())
            for (t0, tw) in TQT:
                o0 = otps.tile([65, 512], F32, name="ot0")
                o1 = otps.tile([65, 512], F32, name="ot1")
                for it in range(NTK):
                    sp = scps.tile([128, 2, 512], F32, name="sc")
                    nc.tensor.matmul(
                        sp[:, 0, :tw],
                        khT[0:64, j, it * 128:(it + 1) * 128],
                        qhT[0:64, j, t0:t0 + tw],
                        start=True, stop=True,
                    )
                    nc.tensor.matmul(
                        sp[:, 1, :tw],
                        khT[64:128, j, it * 128:(it + 1) * 128],
                        qhT[64:128, j, t0:t0 + tw],
                        start=True, stop=True,
                    )
                    next(nxt, None)
                    next(nxt, None)
                    pr = probs_pool.tile([128, 2, 512], BF16, name="pr")
                    nc.scalar.activation(
                        pr[:, :, :tw], sp[:, :, :tw], Exp,
                        bias=biask_sb[:, it:it + 1], scale=0.125,
                    )
                    nc.tensor.matmul(
                        o0[:, :tw], vh[:, it, 2 * j, :], pr[:, 0, :tw],
                        start=(it == 0), stop=(it == NTK - 1),
                    )
                    nc.tensor.matmul(
                        o1[:, :tw], vh[:, it, 2 * j + 1, :], pr[:, 1, :tw],
                        start=(it == 0), stop=(it == NTK - 1),
                    )
                pk0 = park_pool.tile([65, 512], F32, name="pk")
                nc.vector.tensor_copy(pk0[:, :tw], o0[:, :tw])
                nc.sync.dma_start(out_d[:, 2 * j, t0:t0 + tw], pk0[:, :tw])
                pk1 = park_pool.tile([65, 512], F32, name="pk")
                nc.vector.tensor_copy(pk1[:, :tw], o1[:, :tw])
                nc.sync.dma_start(out_d[:, 2 * j + 1, t0:t0 + tw], pk1[:, :tw])
            drain(nxt)

    nc.finalize()
    return nc


def kernel(q, k, v, Wq, bq, Wk, bk, Wv, bv, mask_attn, mask_out):
    q = np.asarray(q, np.float32)
    k = np.asarray(k, np.float32)
    v = np.asarray(v, np.float32)
    Wq = np.asarray(Wq, np.float32)
    Wk = np.asarray(Wk, np.float32)
    Wv = np.asarray(Wv, np.float32)
    bq = np.asarray(bq, np.float32)
    bk = np.asarray(bk, np.float32)
    bv = np.asarray(bv, np.float32)
    mask_attn = np.asarray(mask_attn)
    mask_out = np.asarray(mask_out)

    B, T, _ = q.shape
    idxk = [np.flatnonzero(mask_attn[b]) for b in range(B)]
    idxq = [np.flatnonzero(mask_out[b]) for b in range(B)]
    TK = max(128, -(-max(len(i) for i in idxk) // 128) * 128)
    TQ = max(128, -(-max(len(i) for i in idxq) // 128) * 128)
    NTK = TK // 128

    nc = _build(TQ, TK)

    in_maps = []
    for c in range(N_CORES):
        b, g = c // 2, c % 2
        sl = slice(g * FO, (g + 1) * FO)
        nk, nq = len(idxk[b]), len(idxq[b])
        qc = np.zeros((TQ, HID), np.float32)
        qc[:nq] = q[b][idxq[b]]
        kc = np.zeros((TK, HID), np.float32)
        kc[:nk] = k[b][idxk[b]]
        vc = np.zeros((TK, HID), np.float32)
        vc[:nk] = v[b][idxk[b]]
        biask = np.full(TK, -30000.0, np.float32)
        biask[:nk] = 0.0
        in_maps.append({
            "qT": np.ascontiguousarray(qc.T).astype(ml_dtypes.bfloat16),
            "kT": np.ascontiguousarray(kc.T).astype(ml_dtypes.bfloat16),
            "vT": np.ascontiguousarray(vc.T).astype(ml_dtypes.bfloat16),
            "wqT": np.ascontiguousarray(Wq[sl].T).astype(ml_dtypes.bfloat16),
            "wkT": np.ascontiguousarray(Wk[sl].T).astype(ml_dtypes.bfloat16),
            "wvT": np.ascontiguousarray(Wv[sl].T).astype(ml_dtypes.bfloat16),
            "bq": bq[sl].reshape(1, FO).astype(ml_dtypes.bfloat16),
            "bk": bk[sl].reshape(1, FO).astype(ml_dtypes.bfloat16),
            "bv": bv[sl].reshape(1, FO).astype(ml_dtypes.bfloat16),
            "biask": np.ascontiguousarray(biask.reshape(NTK, 128).T),
            "ones1": np.ones((1, 512), ml_dtypes.bfloat16),
            "onesv": np.ones((128, NTK * HPC), ml_dtypes.bfloat16),
        })

    trace_dir = os.environ.get("KERNEL_TRACE_DIR")
    if trace_dir:
        res = run_bass_kernel_spmd(
            nc, in_maps, list(range(N_CORES)), trace=True, tmpdir=trace_dir
        )
        print(f"HW exec time: {res.exec_time_ns} ns")
    else:
        res = run_bass_kernel_spmd(nc, in_maps, list(range(N_CORES)))

    out_full = np.zeros((B, T, HID), np.float32)
    for c in range(N_CORES):
        b, g = c // 2, c % 2
        nq = len(idxq[b])
        u = res.results[c]["out"]  # [65, HPC, TQ]
        o = u[:64, :, :nq] / u[64:65, :, :nq]
        o = o.transpose(2, 1, 0).reshape(nq, FO)
        out_full[b, idxq[b], g * FO:(g + 1) * FO] = o
    return out_full


# revision 6
# speedup vs baseline: 1.4323x; 1.0570x over previous
"""Multi-head attention (B=4, T=2048, D=1024, H=16) on 8 Trainium2 cores.

Sharding: core c handles (batch b = c//2, head-group g = c%2) — 8 heads,
512 output features. No inter-core communication.

Host-side: rows of K/V masked out by mask_attn and rows of Q masked out by
mask_out are compacted away (their probabilities / outputs are exactly zero
in the reference), then padded to a multiple of 128. Activations and
weight slices are pre-transposed so every device matmul contracts over the
partition dim, and converted to bf16 (PSUM accumulation is fp32).

Device per core: project K/Q into transposed [feature, token] layout and V
into natural [token, feature] layout (biases added via K=1 ones-matmuls);
scores^T = K_h @ Q_h^T per head pair, packed into disjoint PE row groups;
one ScalarE instruction applies scale + key-padding bias + exp per 2-head
PSUM tile; PV accumulates [V_h | 1]^T @ probs^T giving the output and the
softmax denominator (ones column). Projections of head-pair j+1 are
emission-interleaved into pair j's ACT-bound attention loop to fill PE
idle slots. Host divides by the denominator and scatters rows.
"""

import itertools
import os
import sys

sys.path.insert(0, "/opt/trn_rl_repo")

import numpy as np
import ml_dtypes
from contextlib import ExitStack

import concourse.bacc as bacc
import concourse.tile as tile
from concourse import mybir
from concourse.bass_utils import run_bass_kernel_spmd

F32 = mybir.dt.float32
BF16 = mybir.dt.bfloat16

HID = 1024
FO = 512          # projection features per core = 8 heads * 64
HPC = 8           # heads per core
NFI = HID // 128  # contraction chunks
N_CORES = 8


def _tiles(total, w):
    out = []
    o = 0
    while o < total:
        tw = min(w, total - o)
        out.append((o, tw))
        o += tw
    return out


def _build(TQ, TK):
    NTK = TK // 128
    TQT = _tiles(TQ, 512)

    nc = bacc.Bacc("TRN2", target_bir_lowering=False, debug=False)

    qT_d = nc.declare_dram_parameter("qT", [HID, TQ], BF16, isOutput=False)
    kT_d = nc.declare_dram_parameter("kT", [HID, TK], BF16, isOutput=False)
    vT_d = nc.declare_dram_parameter("vT", [HID, TK], BF16, isOutput=False)
    wqT_d = nc.declare_dram_parameter("wqT", [HID, FO], BF16, isOutput=False)
    wkT_d = nc.declare_dram_parameter("wkT", [HID, FO], BF16, isOutput=False)
    wvT_d = nc.declare_dram_parameter("wvT", [HID, FO], BF16, isOutput=False)
    bq_d = nc.declare_dram_parameter("bq", [1, FO], BF16, isOutput=False)
    bk_d = nc.declare_dram_parameter("bk", [1, FO], BF16, isOutput=False)
    bv_d = nc.declare_dram_parameter("bv", [1, FO], BF16, isOutput=False)
    biask_d = nc.declare_dram_parameter("biask", [128, NTK], F32, isOutput=False)
    ones1_d = nc.declare_dram_parameter("ones1", [1, 512], BF16, isOutput=False)
    onesv_d = nc.declare_dram_parameter("onesv", [128, NTK * HPC], BF16, isOutput=False)
    out_d = nc.declare_dram_parameter("out", [65, HPC, TQ], F32, isOutput=True)

    Exp = mybir.ActivationFunctionType.Exp

    with tile.TileContext(nc) as tc, ExitStack() as ctx:
        res = ctx.enter_context(tc.tile_pool(name="res", bufs=1))
        qhT = res.tile([128, 4, TQ], BF16)        # [fo%128, pair, t]
        khT = res.tile([128, 4, TK], BF16)
        vh = res.tile([128, NTK, HPC, 65], BF16)  # [t%128, t//128, head, dh+1]
        ones = res.tile([1, 512], BF16)
        biask_sb = res.tile([128, NTK], F32)
        bq_sb = res.tile([1, FO], BF16)
        bk_sb = res.tile([1, FO], BF16)
        bv_sb = res.tile([1, FO], BF16)
        kT_sb = res.tile([128, NFI, TK], BF16)
        qT_sb = res.tile([128, NFI, TQ], BF16)
        vT_sb = res.tile([128, NFI, TK], BF16)
        wq_sb = res.tile([128, NFI, FO], BF16)
        wk_sb = res.tile([128, NFI, FO], BF16)
        wv_sb = res.tile([128, NFI, FO], BF16)

        # Constants + V-path via gpsimd SWDGE; K-path on the sync ring;
        # vT/qT on the scalar HWDGE ring (idle until attention starts).
        nc.gpsimd.dma_start(biask_sb[:], biask_d[:])
        nc.gpsimd.dma_start(bk_sb[:], bk_d[:])
        nc.gpsimd.dma_start(bq_sb[:], bq_d[:])
        nc.gpsimd.dma_start(bv_sb[:], bv_d[:])
        nc.gpsimd.dma_start(ones[:], ones1_d[:])
        nc.gpsimd.dma_start(vh[:, :, :, 64:65], onesv_d[:])
        nc.sync.dma_start(wk_sb[:], wkT_d.rearrange("(c p) n -> p c n", p=128))
        nc.sync.dma_start(kT_sb[:], kT_d.rearrange("(c p) t -> p c t", p=128))
        nc.sync.dma_start(vT_sb[:], vT_d.rearrange("(c p) t -> p c t", p=128))
        nc.sync.dma_start(wv_sb[:], wvT_d.rearrange("(c p) n -> p c n", p=128))
        nc.sync.dma_start(qT_sb[:], qT_d.rearrange("(c p) t -> p c t", p=128))
        nc.sync.dma_start(wq_sb[:], wqT_d.rearrange("(c p) n -> p c n", p=128))

        ppj = ctx.enter_context(tc.tile_pool(name="ppj", bufs=2, space="PSUM"))
        scps = ctx.enter_context(tc.tile_pool(name="scps", bufs=2, space="PSUM"))
        otps = ctx.enter_context(tc.tile_pool(name="otps", bufs=1, space="PSUM"))
        probs_pool = ctx.enter_context(tc.tile_pool(name="probs", bufs=3))
        park_pool = ctx.enter_context(tc.tile_pool(name="park", bufs=4))

        def gen_kq_proj(jf, src_sb, w_sb, b_sb, dst, nT):
            """Projection of feature tile jf (one head pair), [fo, t] layout."""
            for (t0, tw) in _tiles(nT, 512):
                ps = ppj.tile([128, 512], F32, name="pjps")
                for c in range(NFI):
                    nc.tensor.matmul(
                        ps[:, :tw],
                        w_sb[:, c, jf * 128:(jf + 1) * 128],
                        src_sb[:, c, t0:t0 + tw],
                        start=(c == 0), stop=False,
                    )
                    if c % 3 == 2:
                        yield
                nc.tensor.matmul(
                    ps[:, :tw],
                    b_sb[0:1, jf * 128:(jf + 1) * 128],
                    ones[0:1, :tw],
                    start=False, stop=True,
                )
                nc.vector.tensor_copy(dst[:, jf, t0:t0 + tw], ps[:, :tw])
                yield

        def gen_v_proj(half):
            """V projection for heads 4*half .. 4*half+3, natural layout."""
            f0 = half * 256
            for it in range(NTK):
                ps = ppj.tile([128, 512], F32, name="pjps")
                for c in range(NFI):
                    nc.tensor.matmul(
                        ps[:, :256],
                        vT_sb[:, c, it * 128:(it + 1) * 128],
                        wv_sb[:, c, f0:f0 + 256],
                        start=(c == 0), stop=False,
                    )
                    if c % 4 == 3:
                        yield
                nc.tensor.matmul(
                    ps[:, :256], ones[0:1, 0:128], bv_sb[0:1, f0:f0 + 256],
                    start=False, stop=True,
                )
                nc.vector.tensor_copy(
                    vh[:, it, 4 * half:4 * half + 4, 0:64],
                    ps[:, :256].rearrange("p (h d) -> p h d", h=4),
                )
                yield

        def drain(g):
            for _ in g:
                pass

        # Upfront: everything head pairs 0 (K, Q) and 0-1 (V) need.
        drain(gen_kq_proj(0, kT_sb, wk_sb, bk_sb, khT, TK))
        drain(gen_v_proj(0))
        drain(gen_kq_proj(0, qT_sb, wq_sb, bq_sb, qhT, TQ))

        # Flattened attention pipeline over (pair, tq-tile, tk) slots.
        # PV of slot i is emitted after scores of slot i+1, so the in-order
        # PE never parks behind a PV that waits on the ACT output.
        slots = [
            (j, ti, t0, tw, it)
            for j in range(4)
            for ti, (t0, tw) in enumerate(TQT)
            for it in range(NTK)
        ]
        pair_gens = {}
        for j in range(3):
            gens = [
                gen_kq_proj(j + 1, kT_sb, wk_sb, bk_sb, khT, TK),
                gen_kq_proj(j + 1, qT_sb, wq_sb, bq_sb, qhT, TQ),
            ]
            if j == 1:
                gens.insert(0, gen_v_proj(1))
            pair_gens[j] = itertools.chain(*gens)
        pair_gens[3] = iter(())

        otiles = {}
        prev = None

        cur_o = {}

        def emit_pv(slot):
            j, ti, t0, tw, it = slot
            pr = otiles.pop((j, ti, it))
            o0, o1 = cur_o[(j, ti)]
            nc.tensor.matmul(
                o0[:, :tw], vh[:, it, 2 * j, :], pr[:, 0, :tw],
                start=(it == 0), stop=(it == NTK - 1),
            )
            nc.tensor.matmul(
                o1[:, :tw], vh[:, it, 2 * j + 1, :], pr[:, 1, :tw],
                start=(it == 0), stop=(it == NTK - 1),
            )
            if it == NTK - 1:
                del cur_o[(j, ti)]
                pk0 = park_pool.tile([65, 512], F32, name="pk")
                nc.vector.tensor_copy(pk0[:, :tw], o0[:, :tw])
                nc.sync.dma_start(out_d[:, 2 * j, t0:t0 + tw], pk0[:, :tw])
                pk1 = park_pool.tile([65, 512], F32, name="pk")
                nc.vector.tensor_copy(pk1[:, :tw], o1[:, :tw])
                nc.sync.dma_start(out_d[:, 2 * j + 1, t0:t0 + tw], pk1[:, :tw])

        cur_pair = 0
        for slot in slots:
            j, ti, t0, tw, it = slot
            if j != cur_pair:
                # everything pair j needs must be emitted before its scores
                drain(pair_gens[cur_pair])
                cur_pair = j
            sp = scps.tile([128, 2, 512], F32, name="sc")
            nc.tensor.matmul(
                sp[:, 0, :tw],
                khT[0:64, j, it * 128:(it + 1) * 128],
                qhT[0:64, j, t0:t0 + tw],
                start=True, stop=True,
            )
            nc.tensor.matmul(
                sp[:, 1, :tw],
                khT[64:128, j, it * 128:(it + 1) * 128],
                qhT[64:128, j, t0:t0 + tw],
                start=True, stop=True,
            )
            next(pair_gens[j], None)
            next(pair_gens[j], None)
            pr = probs_pool.tile([128, 2, 512], BF16, name="pr")
            nc.scalar.activation(
                pr[:, :, :tw], sp[:, :, :tw], Exp,
                bias=biask_sb[:, it:it + 1], scale=0.125,
            )
            if it == 0:
                o0 = otps.tile([65, 512], F32, name="ot0")
                o1 = otps.tile([65, 512], F32, name="ot1")
                cur_o[(j, ti)] = (o0, o1)
            otiles[(j, ti, it)] = pr
            if prev is not None:
                emit_pv(prev)
            prev = slot
        emit_pv(prev)
        drain(pair_gens[3])

    nc.finalize()
    return nc


def kernel(q, k, v, Wq, bq, Wk, bk, Wv, bv, mask_attn, mask_out):
    q = np.asarray(q, np.float32)
    k = np.asarray(k, np.float32)
    v = np.asarray(v, np.float32)
    Wq = np.asarray(Wq, np.float32)
    Wk = np.asarray(Wk, np.float32)
    Wv = np.asarray(Wv, np.float32)
    bq = np.asarray(bq, np.float32)
    bk = np.asarray(bk, np.float32)
    bv = np.asarray(bv, np.float32)
    mask_attn = np.asarray(mask_attn)
    mask_out = np.asarray(mask_out)

    B, T, _ = q.shape
    idxk = [np.flatnonzero(mask_attn[b]) for b in range(B)]
    idxq = [np.flatnonzero(mask_out[b]) for b in range(B)]
    TK = max(128, -(-max(len(i) for i in idxk) // 128) * 128)
    TQ = max(128, -(-max(len(i) for i in idxq) // 128) * 128)
    NTK = TK // 128

    nc = _build(TQ, TK)

    in_maps = []
    for c in range(N_CORES):
        b, g = c // 2, c % 2
        sl = slice(g * FO, (g + 1) * FO)
        nk, nq = len(idxk[b]), len(idxq[b])
        qc = np.zeros((TQ, HID), np.float32)
        qc[:nq] = q[b][idxq[b]]
        kc = np.zeros((TK, HID), np.float32)
        kc[:nk] = k[b][idxk[b]]
        vc = np.zeros((TK, HID), np.float32)
        vc[:nk] = v[b][idxk[b]]
        biask = np.full(TK, -30000.0, np.float32)
        biask[:nk] = 0.0
        in_maps.append({
            "qT": np.ascontiguousarray(qc.T).astype(ml_dtypes.bfloat16),
            "kT": np.ascontiguousarray(kc.T).astype(ml_dtypes.bfloat16),
            "vT": np.ascontiguousarray(vc.T).astype(ml_dtypes.bfloat16),
            "wqT": np.ascontiguousarray(Wq[sl].T).astype(ml_dtypes.bfloat16),
            "wkT": np.ascontiguousarray(Wk[sl].T).astype(ml_dtypes.bfloat16),
            "wvT": np.ascontiguousarray(Wv[sl].T).astype(ml_dtypes.bfloat16),
            "bq": bq[sl].reshape(1, FO).astype(ml_dtypes.bfloat16),
            "bk": bk[sl].reshape(1, FO).astype(ml_dtypes.bfloat16),
            "bv": bv[sl].reshape(1, FO).astype(ml_dtypes.bfloat16),
            "biask": np.ascontiguousarray(biask.reshape(NTK, 128).T),
            "ones1": np.ones((1, 512), ml_dtypes.bfloat16),
            "onesv": np.ones((128, NTK * HPC), ml_dtypes.bfloat16),
        })

    trace_dir = os.environ.get("KERNEL_TRACE_DIR")
    if trace_dir:
        res = run_bass_kernel_spmd(
            nc, in_maps, list(range(N_CORES)), trace=True, tmpdir=trace_dir
        )
        print(f"HW exec time: {res.exec_time_ns} ns")
    else:
        res = run_bass_kernel_spmd(nc, in_maps, list(range(N_CORES)))

    out_full = np.zeros((B, T, HID), np.float32)
    for c in range(N_CORES):
        b, g = c // 2, c % 2
        nq = len(idxq[b])
        u = res.results[c]["out"]  # [65, HPC, TQ]
        o = u[:64, :, :nq] / u[64:65, :, :nq]
        o = o.transpose(2, 1, 0).reshape(nq, FO)
        out_full[b, idxq[b], g * FO:(g + 1) * FO] = o
    return out_full
